# revision 18
# baseline (speedup 1.0000x reference)
"""DigitCaps dynamic-routing kernel for 8 Trainium2 NeuronCores.

Strategy: shard the num_route_nodes axis (R=2048 -> 256 per core).
  - Phase 1: u_hat production. Per route r: u[b, (c,m)] = xT_r[k,b].T @ w_r[k,(c,m)]
    on the tensor engine (fp32). u staged in device DRAM; the first routing
    iteration (c uniform = 1/CAPS) is fused in as a running sum over routes.
  - Phase 2: each remaining routing iteration is ONE streaming pass over u:
    per r-tile: dot = sum_m u*v  ->  b_logits += dot -> softmax over caps
    (tile-local) -> s_partial += sum_r c*u.  s is AllReduced across cores
    (contraction over r spans cores), squash computed redundantly per core.

Inputs are sharded host-side: x -> xT[k, r_loc, b] slices, w -> w[r_loc, k, c, m]
slices (transpose is layout prep for DMA/matmul efficiency; all FLOPs on device).
"""

import os
import sys

if "/opt/trn_rl_repo" not in sys.path:
    sys.path.insert(0, "/opt/trn_rl_repo")

import numpy as np

B, R, K, C, M = 128, 2048, 64, 32, 32
CM = C * M
N_CORES = 8
R_LOC = R // N_CORES
RT1 = int(os.environ.get("DC_RT1", "8"))   # routes per tile, u-production
RT2 = int(os.environ.get("DC_RT2", "16"))  # routes per tile, routing passes
S1_ON_PE = os.environ.get("DC_S1PE", "1") == "1"

PROD_ENGINE = os.environ.get("DC_PROD", "vector")   # "vector" | "gpsimd"
U_DT = os.environ.get("DC_U_DT", "float16")         # staged-u dtype
MM_DT = os.environ.get("DC_MM", "float32r")         # matmul input dtype

_compiled = {}
LAST_RESULT = None          # BassKernelResults of the most recent run (for test.py)


def _view(ap, dims):
    """Free-dim view of an AP: keep its partition dim, replace free dims by
    [step, count] pairs (element steps). step 0 = broadcast."""
    import concourse.bass as bass

    return bass.AP(
        tensor=ap.tensor,
        offset=ap.offset,
        ap=[list(ap.ap[0])] + [[s, c] for s, c in dims],
    )


def _ap(ap, dims):
    """Fully custom AP (all dims given) at the base offset of `ap`."""
    import concourse.bass as bass

    return bass.AP(
        tensor=ap.tensor,
        offset=ap.offset,
        ap=[[s, c] for s, c in dims],
    )


def _squash(nc, pool, s_ap, v_ap):
    """v = s * |s|^2 / ((1 + |s|^2) (sqrt(|s|^2) + 1e-8)), norm over m."""
    import concourse.mybir as mybir

    f32 = mybir.dt.float32
    op = mybir.AluOpType
    sq_full = pool.tile([B, CM], f32, tag="sq_full")
    nc.vector.tensor_tensor(sq_full[:], s_ap, s_ap, op=op.mult)
    sq = pool.tile([B, C], f32, tag="sq")
    nc.vector.tensor_reduce(
        sq[:], _view(sq_full[:], [(1, C), (C, M)]), axis=mybir.AxisListType.X,
        op=op.add)
    rt = pool.tile([B, C], f32, tag="rt")
    nc.scalar.activation(rt[:], sq[:], mybir.ActivationFunctionType.Sqrt)
    nc.vector.tensor_scalar(rt[:], rt[:], 1e-8, None, op0=op.add)
    den = pool.tile([B, C], f32, tag="den")
    nc.vector.tensor_scalar(den[:], sq[:], 1.0, None, op0=op.add)
    nc.vector.tensor_tensor(den[:], den[:], rt[:], op=op.mult)
    fi = pool.tile([B, C], f32, tag="fi")
    nc.vector.reciprocal(fi[:], den[:])
    nc.vector.tensor_tensor(fi[:], fi[:], sq[:], op=op.mult)
    # v = s * f (f broadcast over m)
    nc.vector.tensor_tensor(
        v_ap,
        _view(s_ap, [(C, M), (1, C)]),
        _view(fi[:], [(0, M), (1, C)]),
        op=op.mult,
    )


def _build(n_iters, repeat=1):
    import concourse.mybir as mybir
    import concourse.tile as tile
    from concourse import bacc

    f32 = mybir.dt.float32
    u_dt = getattr(mybir.dt, U_DT)
    mm_dt = getattr(mybir.dt, MM_DT)
    op = mybir.AluOpType
    AX = mybir.AxisListType

    nc = bacc.Bacc("TRN2", target_bir_lowering=False, debug=False,
                   num_devices=N_CORES)
    xT = nc.dram_tensor("xT", [K, R_LOC, B], mm_dt, kind="ExternalInput").ap()
    wT = nc.dram_tensor("wT", [R_LOC, K, C, M], mm_dt,
                        kind="ExternalInput").ap()
    out = nc.dram_tensor("out", [B, CM], f32, kind="ExternalOutput").ap()

    if PROD_ENGINE == "split":
        prod_p, prod_q = nc.vector, nc.gpsimd
    else:
        prod_p = prod_q = {"gpsimd": nc.gpsimd, "vector": nc.vector}[PROD_ENGINE]

    with tile.TileContext(nc) as tc:
        with (
            tc.tile_pool(name="sm", bufs=2) as sm,       # small temps
            tc.tile_pool(name="persist", bufs=1) as persist,
            tc.tile_pool(name="dram", bufs=1, space="DRAM") as dram,
            tc.tile_pool(name="drbounce", bufs=min(2 * n_iters * repeat, 8),
                         space="DRAM") as drb,
        ):
            u_dram = dram.tile([B, R_LOC * CM], u_dt)
            b_log = persist.tile([B, R_LOC * C], f32)   # logits, layout (r, c)
            v_sb = persist.tile([B, CM], f32)           # current v (fp32)

            def allreduce_squash(s_acc_tile, scale):
                bin_ = drb.tile([B, CM], f32, tag="bin")
                bout = drb.tile([B, CM], f32, tag="bout")
                nc.sync.dma_start(bin_[:], s_acc_tile[:])
                nc.gpsimd.collective_compute(
                    "AllReduce", op.add,
                    replica_groups=[list(range(N_CORES))],
                    ins=[bin_.opt()], outs=[bout.opt()],
                )
                s_sb = sm.tile([B, CM], f32, tag="s_sb")
                nc.sync.dma_start(s_sb[:], bout[:])
                if scale != 1.0:
                    nc.vector.tensor_scalar(s_sb[:], s_sb[:], scale, None,
                                            op0=op.mult)
                _squash(nc, sm, s_sb[:], v_sb[:])

            def emit_phase1_packed():
                """u production with route-pairs packed on 128 partitions;
                iteration-1 s accumulated on the PE in a dedicated PSUM pair
                via K=128 packed matmuls (u_r0 + u_r1 per pair)."""
                s_acc = sm.tile([B, CM], f32, tag="s_acc")
                n_tiles = R_LOC // RT1
                half = RT1 // 2
                with (
                    tc.tile_pool(name="xp", bufs=3) as xp,
                    tc.tile_pool(name="wp", bufs=3) as wp,
                    tc.tile_pool(name="up1", bufs=3) as up1,
                    tc.tile_pool(name="pp", bufs=3, space="PSUM") as pp,
                    tc.tile_pool(name="s1p", bufs=1, space="PSUM") as s1p,
                ):
                    s1_psum = s1p.tile([B, CM], f32)
                    for t in range(n_tiles):
                        xt = xp.tile([2 * K, half * B], mm_dt)
                        # partitions 0..63 <- even routes' k, 64..127 <- odd
                        nc.sync.dma_start(
                            xt[0:K, :],
                            _ap(xT[:, t * RT1:(t + 1) * RT1, :],
                                [(R_LOC * B, K), (2 * B, half), (1, B)]))
                        nc.sync.dma_start(
                            xt[K:2 * K, :],
                            _ap(xT[:, t * RT1 + 1:(t + 1) * RT1, :],
                                [(R_LOC * B, K), (2 * B, half), (1, B)]))
                        wt = wp.tile([2 * K, half * CM], mm_dt)
                        nc.sync.dma_start(
                            wt[0:K, :],
                            _ap(wT[t * RT1:(t + 1) * RT1],
                                [(CM, K), (2 * K * CM, half), (1, CM)]))
                        nc.sync.dma_start(
                            wt[K:2 * K, :],
                            _ap(wT[t * RT1 + 1:(t + 1) * RT1],
                                [(CM, K), (2 * K * CM, half), (1, CM)]))
                        ut = up1.tile([B, RT1 * CM], u_dt)
                        for rp in range(half):
                            first = (t == 0 and rp == 0)
                            last = (t == n_tiles - 1 and rp == half - 1)
                            for h in range(2):
                                # packed: u_even + u_odd accumulated into s1
                                nc.tensor.matmul(
                                    s1_psum[:, h * 512:(h + 1) * 512],
                                    xt[:, rp * B:(rp + 1) * B],
                                    wt[:, rp * CM + h * 512:
                                       rp * CM + (h + 1) * 512],
                                    start=first, stop=last,
                                )
                            for par in range(2):
                                j = 2 * rp + par
                                ps = pp.tile([B, CM], f32)
                                for h in range(2):
                                    nc.tensor.matmul(
                                        ps[:, h * 512:(h + 1) * 512],
                                        xt[par * K:(par + 1) * K,
                                           rp * B:(rp + 1) * B],
                                        wt[par * K:(par + 1) * K,
                                           rp * CM + h * 512:
                                           rp * CM + (h + 1) * 512],
                                        start=True, stop=True,
                                    )
                                nc.scalar.copy(
                                    ut[:, j * CM:(j + 1) * CM], ps[:])
                        nc.sync.dma_start(
                            u_dram[:, t * RT1 * CM:(t + 1) * RT1 * CM], ut[:])
                    nc.vector.tensor_copy(s_acc[:], s1_psum[:])
                return s_acc

            def emit_phase1_plain():
                s_acc = sm.tile([B, CM], f32, tag="s_acc")
                nc.vector.memset(s_acc[:], 0.0)
                with (
                    tc.tile_pool(name="xp", bufs=3) as xp,
                    tc.tile_pool(name="wp", bufs=3) as wp,
                    tc.tile_pool(name="up1", bufs=3) as up1,
                    tc.tile_pool(name="pp", bufs=4, space="PSUM") as pp,
                ):
                    for t in range(R_LOC // RT1):
                        xt = xp.tile([K, RT1 * B], mm_dt)
                        nc.sync.dma_start(xt[:],
                                          xT[:, t * RT1:(t + 1) * RT1, :])
                        wt = wp.tile([K, RT1 * CM], mm_dt)
                        # src order matches dst [k partition, (r_t, cm) free];
                        # wT layout [r, k, c, m]
                        nc.sync.dma_start(
                            wt[:],
                            _ap(wT[t * RT1:(t + 1) * RT1],
                                [(CM, K), (K * CM, RT1), (1, CM)]),
                        )
                        ut = up1.tile([B, RT1 * CM], u_dt)
                        for j in range(RT1):
                            ps = pp.tile([B, CM], f32)
                            for h in range(2):
                                nc.tensor.matmul(
                                    ps[:, h * 512:(h + 1) * 512],
                                    xt[:, j * B:(j + 1) * B],
                                    wt[:, j * CM + h * 512:
                                       j * CM + (h + 1) * 512],
                                    start=True, stop=True,
                                )
                            nc.scalar.copy(ut[:, j * CM:(j + 1) * CM], ps[:])
                        # s1 partial: sum over the tile's routes
                        red = sm.tile([B, CM], f32, tag="red")
                        nc.vector.tensor_reduce(
                            red[:], _view(ut[:], [(1, CM), (CM, RT1)]),
                            axis=AX.X, op=op.add)
                        nc.vector.tensor_tensor(s_acc[:], s_acc[:], red[:],
                                                op=op.add)
                        nc.sync.dma_start(
                            u_dram[:, t * RT1 * CM:(t + 1) * RT1 * CM], ut[:])
                return s_acc

            def emit_once():
                # ------------- Phase 1: u production + iteration-1 s ---------
                if S1_ON_PE:
                    s_acc = emit_phase1_packed()
                else:
                    s_acc = emit_phase1_plain()

                allreduce_squash(s_acc, 1.0 / C)

                # ------------- Phase 2: remaining routing iterations ---------
                with (
                    tc.tile_pool(name="up2", bufs=2) as up2,
                    tc.tile_pool(name="pq",
                                 bufs=int(os.environ.get("DC_PQB", "2"))) as pq,
                    tc.tile_pool(name="vxp", bufs=1) as vxp,
                    tc.tile_pool(name="cxp", bufs=2) as cxp,
                ):
                    for it in range(2, n_iters + 1):
                        s_acc = sm.tile([B, CM], f32, tag="s_acc")
                        nc.vector.memset(s_acc[:], 0.0)
                        # v expanded over the tile's routes (ACT, once/pass)
                        v_exp = vxp.tile([B, RT2 * CM], u_dt, tag="v_exp")
                        nc.scalar.copy(
                            v_exp[:], _view(v_sb[:], [(0, RT2), (1, CM)]))
                        for t in range(R_LOC // RT2):
                            ut = up2.tile([B, RT2 * CM], u_dt)
                            nc.sync.dma_start(
                                ut[:],
                                u_dram[:, t * RT2 * CM:(t + 1) * RT2 * CM])
                            # p = u * v  (both contiguous -> DVE 2x mode)
                            p = pq.tile([B, RT2 * CM], u_dt, tag="pq")
                            prod_p.tensor_tensor(p[:], ut[:], v_exp[:],
                                                 op=op.mult)
                            # dot[b, (r_t, c)] = sum_m p
                            dot = sm.tile([B, RT2 * C], f32, tag="dot")
                            nc.vector.tensor_reduce(
                                dot[:],
                                _view(p[:], [(CM, RT2), (1, C), (C, M)]),
                                axis=AX.X, op=op.add)
                            blt = b_log[:, t * RT2 * C:(t + 1) * RT2 * C]
                            if it == 2:
                                nc.vector.tensor_copy(blt, dot[:])
                            else:
                                nc.vector.tensor_tensor(blt, blt, dot[:],
                                                        op=op.add)
                            # softmax over caps (innermost c of blt)
                            mx = sm.tile([B, RT2], f32, tag="mx")
                            nc.vector.tensor_reduce(
                                mx[:], _view(blt, [(C, RT2), (1, C)]),
                                axis=AX.X, op=op.max)
                            e = sm.tile([B, RT2 * C], f32, tag="e")
                            nc.vector.tensor_tensor(
                                _view(e[:], [(C, RT2), (1, C)]),
                                _view(blt, [(C, RT2), (1, C)]),
                                _view(mx[:], [(1, RT2), (0, C)]),
                                op=op.subtract)
                            nc.scalar.activation(
                                e[:], e[:], mybir.ActivationFunctionType.Exp)
                            z = sm.tile([B, RT2], f32, tag="z")
                            nc.vector.tensor_reduce(
                                z[:], _view(e[:], [(C, RT2), (1, C)]),
                                axis=AX.X, op=op.add)
                            nc.vector.reciprocal(z[:], z[:])
                            coef = sm.tile([B, RT2 * C], u_dt, tag="coef")
                            nc.vector.tensor_tensor(
                                _view(coef[:], [(C, RT2), (1, C)]),
                                _view(e[:], [(C, RT2), (1, C)]),
                                _view(z[:], [(1, RT2), (0, C)]),
                                op=op.mult)
                            # coef expanded over m on ACT, then plain 2x TT
                            coef_exp = cxp.tile([B, RT2 * CM], u_dt,
                                                tag="coef_exp")
                            nc.scalar.copy(
                                coef_exp[:],
                                _view(coef[:], [(C, RT2), (0, M), (1, C)]))
                            q = pq.tile([B, RT2 * CM], u_dt, tag="pq")
                            prod_q.tensor_tensor(q[:], ut[:], coef_exp[:],
                                                 op=op.mult)
                            # s partial += sum over r_t of q
                            red = sm.tile([B, CM], f32, tag="red")
                            nc.vector.tensor_reduce(
                                red[:],
                                _view(q[:], [(C, M), (1, C), (CM, RT2)]),
                                axis=AX.X, op=op.add)
                            nc.vector.tensor_tensor(s_acc[:], s_acc[:],
                                                    red[:], op=op.add)
                        allreduce_squash(s_acc, 1.0)

            for _ in range(repeat):
                emit_once()

            nc.sync.dma_start(out[:], v_sb[:])

    nc.compile()
    return nc


def kernel(x, route_weights, num_iterations):
    global LAST_RESULT
    from concourse import bass_utils

    n = int(num_iterations)
    assert n >= 1
    x = np.asarray(x, dtype=np.float32)
    w = np.asarray(route_weights, dtype=np.float32)
    assert x.shape == (B, R, K) and w.shape == (R, C, K, M)

    if n not in _compiled:
        _compiled[n] = _build(n)
    nc = _compiled[n]

    in_maps = []
    for c in range(N_CORES):
        sl = slice(c * R_LOC, (c + 1) * R_LOC)
        xT_c = np.ascontiguousarray(x[:, sl, :].transpose(2, 1, 0))
        wT_c = np.ascontiguousarray(w[sl].transpose(0, 2, 3, 1))
        in_maps.append({"xT": xT_c, "wT": wT_c})

    res = bass_utils.run_bass_kernel_spmd(
        nc, in_maps, core_ids=list(range(N_CORES)))
    LAST_RESULT = res
    return np.ascontiguousarray(
        res.results[0]["out"].reshape(B, M, C).transpose(0, 2, 1)
    ).astype(np.float32)


# revision 19
# speedup vs baseline: 1.3165x; 1.3165x over previous
"""DigitCaps dynamic-routing kernel for 8 Trainium2 NeuronCores.

Strategy: shard the num_route_nodes axis (R=2048 -> 256 per core).
  - Phase 1: u_hat production. Per route r: u[b, (c,m)] = xT_r[k,b].T @ w_r[k,(c,m)]
    on the tensor engine (fp32). u staged in device DRAM; the first routing
    iteration (c uniform = 1/CAPS) is fused in as a running sum over routes.
  - Phase 2: each remaining routing iteration is ONE streaming pass over u:
    per r-tile: dot = sum_m u*v  ->  b_logits += dot -> softmax over caps
    (tile-local) -> s_partial += sum_r c*u.  s is AllReduced across cores
    (contraction over r spans cores), squash computed redundantly per core.

Inputs are sharded host-side: x -> xT[k, r_loc, b] slices, w -> w[r_loc, k, c, m]
slices (transpose is layout prep for DMA/matmul efficiency; all FLOPs on device).
"""

import os
import sys

if "/opt/trn_rl_repo" not in sys.path:
    sys.path.insert(0, "/opt/trn_rl_repo")

import numpy as np

B, R, K, C, M = 128, 2048, 64, 32, 32
CM = C * M
N_CORES = 8
R_LOC = R // N_CORES
RT1 = int(os.environ.get("DC_RT1", "8"))   # routes per tile, u-production
RT2 = int(os.environ.get("DC_RT2", "16"))  # routes per tile, routing passes
S1_ON_PE = os.environ.get("DC_S1PE", "1") == "1"

PROD_ENGINE = os.environ.get("DC_PROD", "vector")   # "vector" | "gpsimd"
U_DT = os.environ.get("DC_U_DT", "float16")         # staged-u dtype
MM_DT = os.environ.get("DC_MM", "float32r")         # matmul input dtype

_compiled = {}
LAST_RESULT = None          # BassKernelResults of the most recent run (for test.py)


def _view(ap, dims):
    """Free-dim view of an AP: keep its partition dim, replace free dims by
    [step, count] pairs (element steps). step 0 = broadcast."""
    import concourse.bass as bass

    return bass.AP(
        tensor=ap.tensor,
        offset=ap.offset,
        ap=[list(ap.ap[0])] + [[s, c] for s, c in dims],
    )


def _ap(ap, dims):
    """Fully custom AP (all dims given) at the base offset of `ap`."""
    import concourse.bass as bass

    return bass.AP(
        tensor=ap.tensor,
        offset=ap.offset,
        ap=[[s, c] for s, c in dims],
    )


def _squash(nc, pool, s_ap, v_ap):
    """v = s * |s|^2 / ((1 + |s|^2) (sqrt(|s|^2) + 1e-8)), norm over m."""
    import concourse.mybir as mybir

    f32 = mybir.dt.float32
    op = mybir.AluOpType
    sq_full = pool.tile([B, CM], f32, tag="sq_full")
    nc.vector.tensor_tensor(sq_full[:], s_ap, s_ap, op=op.mult)
    sq = pool.tile([B, C], f32, tag="sq")
    nc.vector.tensor_reduce(
        sq[:], _view(sq_full[:], [(1, C), (C, M)]), axis=mybir.AxisListType.X,
        op=op.add)
    rt = pool.tile([B, C], f32, tag="rt")
    nc.scalar.activation(rt[:], sq[:], mybir.ActivationFunctionType.Sqrt)
    nc.vector.tensor_scalar(rt[:], rt[:], 1e-8, None, op0=op.add)
    den = pool.tile([B, C], f32, tag="den")
    nc.vector.tensor_scalar(den[:], sq[:], 1.0, None, op0=op.add)
    nc.vector.tensor_tensor(den[:], den[:], rt[:], op=op.mult)
    fi = pool.tile([B, C], f32, tag="fi")
    nc.vector.reciprocal(fi[:], den[:])
    nc.vector.tensor_tensor(fi[:], fi[:], sq[:], op=op.mult)
    # v = s * f (f broadcast over m)
    nc.vector.tensor_tensor(
        v_ap,
        _view(s_ap, [(C, M), (1, C)]),
        _view(fi[:], [(0, M), (1, C)]),
        op=op.mult,
    )


def _build(n_iters, repeat=1):
    import concourse.mybir as mybir
    import concourse.tile as tile
    from concourse import bacc

    f32 = mybir.dt.float32
    u_dt = getattr(mybir.dt, U_DT)
    mm_dt = getattr(mybir.dt, MM_DT)
    op = mybir.AluOpType
    AX = mybir.AxisListType

    nc = bacc.Bacc("TRN2", target_bir_lowering=False, debug=False,
                   num_devices=N_CORES)
    xT = nc.dram_tensor("xT", [K, R_LOC, B], mm_dt, kind="ExternalInput").ap()
    wT = nc.dram_tensor("wT", [R_LOC, K, C, M], mm_dt,
                        kind="ExternalInput").ap()
    out = nc.dram_tensor("out", [B, CM], f32, kind="ExternalOutput").ap()

    if PROD_ENGINE == "split":
        prod_p, prod_q = nc.vector, nc.gpsimd
    else:
        prod_p = prod_q = {"gpsimd": nc.gpsimd, "vector": nc.vector}[PROD_ENGINE]

    with tile.TileContext(nc) as tc:
        with (
            tc.tile_pool(name="sm", bufs=2) as sm,       # small temps
            tc.tile_pool(name="persist", bufs=1) as persist,
            tc.tile_pool(name="dram", bufs=1, space="DRAM") as dram,
            tc.tile_pool(name="drbounce", bufs=min(2 * n_iters * repeat, 8),
                         space="DRAM") as drb,
        ):
            u_dram = dram.tile([B, R_LOC * CM], u_dt)
            b_log = persist.tile([B, R_LOC * C], f32)   # logits, layout (r, c)
            v_sb = persist.tile([B, CM], f32)           # current v (fp32)

            def allreduce_squash(s_acc_tile, scale):
                bin_ = drb.tile([B, CM], f32, tag="bin")
                bout = drb.tile([B, CM], f32, tag="bout")
                nc.sync.dma_start(bin_[:], s_acc_tile[:])
                nc.gpsimd.collective_compute(
                    "AllReduce", op.add,
                    replica_groups=[list(range(N_CORES))],
                    ins=[bin_.opt()], outs=[bout.opt()],
                )
                s_sb = sm.tile([B, CM], f32, tag="s_sb")
                nc.sync.dma_start(s_sb[:], bout[:])
                if scale != 1.0:
                    nc.vector.tensor_scalar(s_sb[:], s_sb[:], scale, None,
                                            op0=op.mult)
                _squash(nc, sm, s_sb[:], v_sb[:])

            def emit_phase1_packed():
                """u production with route-pairs packed on 128 partitions;
                iteration-1 s accumulated on the PE in a dedicated PSUM pair
                via K=128 packed matmuls (u_r0 + u_r1 per pair)."""
                s_acc = sm.tile([B, CM], f32, tag="s_acc")
                n_tiles = R_LOC // RT1
                half = RT1 // 2
                with (
                    tc.tile_pool(name="xp", bufs=3) as xp,
                    tc.tile_pool(name="wp", bufs=3) as wp,
                    tc.tile_pool(name="up1", bufs=3) as up1,
                    tc.tile_pool(name="pp", bufs=3, space="PSUM") as pp,
                    tc.tile_pool(name="s1p", bufs=1, space="PSUM") as s1p,
                ):
                    s1_psum = s1p.tile([B, CM], f32)
                    for t in range(n_tiles):
                        xt = xp.tile([2 * K, half * B], mm_dt)
                        # partitions 0..63 <- even routes' k, 64..127 <- odd
                        nc.sync.dma_start(
                            xt[0:K, :],
                            _ap(xT[:, t * RT1:(t + 1) * RT1, :],
                                [(R_LOC * B, K), (2 * B, half), (1, B)]))
                        nc.sync.dma_start(
                            xt[K:2 * K, :],
                            _ap(xT[:, t * RT1 + 1:(t + 1) * RT1, :],
                                [(R_LOC * B, K), (2 * B, half), (1, B)]))
                        wt = wp.tile([2 * K, half * CM], mm_dt)
                        nc.sync.dma_start(
                            wt[0:K, :],
                            _ap(wT[t * RT1:(t + 1) * RT1],
                                [(CM, K), (2 * K * CM, half), (1, CM)]))
                        nc.sync.dma_start(
                            wt[K:2 * K, :],
                            _ap(wT[t * RT1 + 1:(t + 1) * RT1],
                                [(CM, K), (2 * K * CM, half), (1, CM)]))
                        ut = up1.tile([B, RT1 * CM], u_dt)
                        for rp in range(half):
                            first = (t == 0 and rp == 0)
                            last = (t == n_tiles - 1 and rp == half - 1)
                            for h in range(2):
                                # packed: u_even + u_odd accumulated into s1
                                nc.tensor.matmul(
                                    s1_psum[:, h * 512:(h + 1) * 512],
                                    xt[:, rp * B:(rp + 1) * B],
                                    wt[:, rp * CM + h * 512:
                                       rp * CM + (h + 1) * 512],
                                    start=first, stop=last,
                                )
                            for par in range(2):
                                j = 2 * rp + par
                                ps = pp.tile([B, CM], f32)
                                for h in range(2):
                                    nc.tensor.matmul(
                                        ps[:, h * 512:(h + 1) * 512],
                                        xt[par * K:(par + 1) * K,
                                           rp * B:(rp + 1) * B],
                                        wt[par * K:(par + 1) * K,
                                           rp * CM + h * 512:
                                           rp * CM + (h + 1) * 512],
                                        start=True, stop=True,
                                    )
                                nc.scalar.copy(
                                    ut[:, j * CM:(j + 1) * CM], ps[:])
                        nc.sync.dma_start(
                            u_dram[:, t * RT1 * CM:(t + 1) * RT1 * CM], ut[:])
                    nc.vector.tensor_copy(s_acc[:], s1_psum[:])
                return s_acc

            def emit_phase1_plain():
                s_acc = sm.tile([B, CM], f32, tag="s_acc")
                nc.vector.memset(s_acc[:], 0.0)
                with (
                    tc.tile_pool(name="xp", bufs=3) as xp,
                    tc.tile_pool(name="wp", bufs=3) as wp,
                    tc.tile_pool(name="up1", bufs=3) as up1,
                    tc.tile_pool(name="pp", bufs=4, space="PSUM") as pp,
                ):
                    for t in range(R_LOC // RT1):
                        xt = xp.tile([K, RT1 * B], mm_dt)
                        nc.sync.dma_start(xt[:],
                                          xT[:, t * RT1:(t + 1) * RT1, :])
                        wt = wp.tile([K, RT1 * CM], mm_dt)
                        # src order matches dst [k partition, (r_t, cm) free];
                        # wT layout [r, k, c, m]
                        nc.sync.dma_start(
                            wt[:],
                            _ap(wT[t * RT1:(t + 1) * RT1],
                                [(CM, K), (K * CM, RT1), (1, CM)]),
                        )
                        ut = up1.tile([B, RT1 * CM], u_dt)
                        for j in range(RT1):
                            ps = pp.tile([B, CM], f32)
                            for h in range(2):
                                nc.tensor.matmul(
                                    ps[:, h * 512:(h + 1) * 512],
                                    xt[:, j * B:(j + 1) * B],
                                    wt[:, j * CM + h * 512:
                                       j * CM + (h + 1) * 512],
                                    start=True, stop=True,
                                )
                            nc.scalar.copy(ut[:, j * CM:(j + 1) * CM], ps[:])
                        # s1 partial: sum over the tile's routes
                        red = sm.tile([B, CM], f32, tag="red")
                        nc.vector.tensor_reduce(
                            red[:], _view(ut[:], [(1, CM), (CM, RT1)]),
                            axis=AX.X, op=op.add)
                        nc.vector.tensor_tensor(s_acc[:], s_acc[:], red[:],
                                                op=op.add)
                        nc.sync.dma_start(
                            u_dram[:, t * RT1 * CM:(t + 1) * RT1 * CM], ut[:])
                return s_acc

            def emit_once():
                # ------------- Phase 1: u production + iteration-1 s ---------
                if S1_ON_PE:
                    s_acc = emit_phase1_packed()
                else:
                    s_acc = emit_phase1_plain()

                allreduce_squash(s_acc, 1.0 / C)

                # ------------- Phase 2: remaining routing iterations ---------
                with (
                    tc.tile_pool(name="up2", bufs=2) as up2,
                    tc.tile_pool(name="pq",
                                 bufs=int(os.environ.get("DC_PQB", "2"))) as pq,
                    tc.tile_pool(name="vxp", bufs=1) as vxp,
                ):
                    for it in range(2, n_iters + 1):
                        s_acc = sm.tile([B, CM], f32, tag="s_acc")
                        nc.vector.memset(s_acc[:], 0.0)
                        # v expanded over the tile's routes (ACT, once/pass)
                        v_exp = vxp.tile([B, RT2 * CM], u_dt, tag="v_exp")
                        nc.scalar.copy(
                            v_exp[:], _view(v_sb[:], [(0, RT2), (1, CM)]))
                        for t in range(R_LOC // RT2):
                            ut = up2.tile([B, RT2 * CM], u_dt)
                            nc.sync.dma_start(
                                ut[:],
                                u_dram[:, t * RT2 * CM:(t + 1) * RT2 * CM])
                            # p = u * v  (both contiguous -> DVE 2x mode)
                            p = pq.tile([B, RT2 * CM], u_dt, tag="pq")
                            prod_p.tensor_tensor(p[:], ut[:], v_exp[:],
                                                 op=op.mult)
                            # dot[b, (r_t, c)] = sum_m p
                            dot = sm.tile([B, RT2 * C], f32, tag="dot")
                            nc.vector.tensor_reduce(
                                dot[:],
                                _view(p[:], [(CM, RT2), (1, C), (C, M)]),
                                axis=AX.X, op=op.add)
                            blt = b_log[:, t * RT2 * C:(t + 1) * RT2 * C]
                            if it == 2:
                                nc.vector.tensor_copy(blt, dot[:])
                            else:
                                nc.vector.tensor_tensor(blt, blt, dot[:],
                                                        op=op.add)
                            # softmax over caps (innermost c of blt)
                            mx = sm.tile([B, RT2], f32, tag="mx")
                            nc.vector.tensor_reduce(
                                mx[:], _view(blt, [(C, RT2), (1, C)]),
                                axis=AX.X, op=op.max)
                            e = sm.tile([B, RT2 * C], f32, tag="e")
                            nc.vector.tensor_tensor(
                                _view(e[:], [(C, RT2), (1, C)]),
                                _view(blt, [(C, RT2), (1, C)]),
                                _view(mx[:], [(1, RT2), (0, C)]),
                                op=op.subtract)
                            nc.scalar.activation(
                                e[:], e[:], mybir.ActivationFunctionType.Exp)
                            z = sm.tile([B, RT2], f32, tag="z")
                            nc.vector.tensor_reduce(
                                z[:], _view(e[:], [(C, RT2), (1, C)]),
                                axis=AX.X, op=op.add)
                            nc.vector.reciprocal(z[:], z[:])
                            coef = sm.tile([B, RT2 * C], u_dt, tag="coef")
                            nc.vector.tensor_tensor(
                                _view(coef[:], [(C, RT2), (1, C)]),
                                _view(e[:], [(C, RT2), (1, C)]),
                                _view(z[:], [(1, RT2), (0, C)]),
                                op=op.mult)
                            # q = u * coef (coef broadcast over m)
                            q = pq.tile([B, RT2 * CM], u_dt, tag="pq")
                            prod_q.tensor_tensor(
                                _view(q[:], [(CM, RT2), (C, M), (1, C)]),
                                _view(ut[:], [(CM, RT2), (C, M), (1, C)]),
                                _view(coef[:], [(C, RT2), (0, M), (1, C)]),
                                op=op.mult)
                            # s partial += sum over r_t of q
                            red = sm.tile([B, CM], f32, tag="red")
                            nc.vector.tensor_reduce(
                                red[:],
                                _view(q[:], [(C, M), (1, C), (CM, RT2)]),
                                axis=AX.X, op=op.add)
                            nc.vector.tensor_tensor(s_acc[:], s_acc[:],
                                                    red[:], op=op.add)
                        allreduce_squash(s_acc, 1.0)

            for _ in range(repeat):
                emit_once()

            nc.sync.dma_start(out[:], v_sb[:])

    nc.compile()
    return nc


def kernel(x, route_weights, num_iterations):
    global LAST_RESULT
    from concourse import bass_utils

    n = int(num_iterations)
    assert n >= 1
    x = np.asarray(x, dtype=np.float32)
    w = np.asarray(route_weights, dtype=np.float32)
    assert x.shape == (B, R, K) and w.shape == (R, C, K, M)

    if n not in _compiled:
        _compiled[n] = _build(n)
    nc = _compiled[n]

    in_maps = []
    for c in range(N_CORES):
        sl = slice(c * R_LOC, (c + 1) * R_LOC)
        xT_c = np.ascontiguousarray(x[:, sl, :].transpose(2, 1, 0))
        wT_c = np.ascontiguousarray(w[sl].transpose(0, 2, 3, 1))
        in_maps.append({"xT": xT_c, "wT": wT_c})

    res = bass_utils.run_bass_kernel_spmd(
        nc, in_maps, core_ids=list(range(N_CORES)))
    LAST_RESULT = res
    return np.ascontiguousarray(
        res.results[0]["out"].reshape(B, M, C).transpose(0, 2, 1)
    ).astype(np.float32)
